# revision 20
# baseline (speedup 1.0000x reference)
"""Single-head causal attention (B=8, S=2048, D=1024, H=64).

Data-parallel over batch: each of the 8 NeuronCores computes one batch
element's full attention head.  Per core:

  qkT  = [32Wq|32Wk]^T @ x^T  -> [128, S] rows 0-63 qT, 64-127 kT
         (fp8e4 DoubleRow: 2 contraction chunks per stream = half the
         matmuls of bf16)
  qkT2 = partition-swapped qkT -> rows 0-63 kT, 64-127 qT, produced by
         two SBUF->SBUF DMAs on the gpsimd SWDGE queue (no PE/DVE cost)
  vT   = Wv^T @ x^T -> bf16, via 2-way col-tiled matmuls that split the
         q (moving) dim: cols 0-255 of each q-block accumulate into
         PSUM rows 0-63 (tile (0,0)), cols 256-511 into rows 64-127
         (tile (0,64)); both streams run concurrently in the PE array
         (halves the PE time of the V projection) and one [128,256]
         cast stores both halves of vT2 per block
  v    = transpose(vT) -> [k, 64] tiles + ones column (PE transpose)
  sT[k,q] = kT_i^T @ qT  -> row-packed strip pairs run concurrently
         (row groups 0-1 vs 2-3), bf16
  p = exp(sT * 2^-15) -> bf16 tiles keyed (qb&1, pair); fully-masked
         rects stay zero, diagonal-tile triangles zeroed by gpsimd
         affine_select; AV runs two pairs behind exp
  oT[65, q] += v_aug[k]^T @ p    (row 64 = Z = sum(exp))
  DMA oT out as bf16; host does out[q,h] = oT[h,q] / Z[q].

Scheduling: input DMAs ride the sync HWDGE queue ordered by consumer
deadline; a short junk-matmul burst at the head keeps the PE HAM
activity window alive across the initial DMA wait and the projection->
swap bubble so the PE un-throttles (K=4/8 -> 8/8) as early as
possible; v-projections/transposes and the block-2/3 projections are
interleaved into the attention pair stream as background work.
"""

import sys

for _p in ("/opt/trn_rl_repo",):
    if _p not in sys.path:
        sys.path.insert(0, _p)

import numpy as np
import ml_dtypes

import concourse.bacc as bacc
import concourse.bass as bass
import concourse.mybir as mybir
from concourse import masks, tile
from concourse.bass_utils import run_bass_kernel_spmd

B, S, D, H = 8, 2048, 1024, 64
P = 128
QB = 512            # q-block width (one PSUM bank of f32)
NB = S // QB        # 4 q-blocks
KT = S // P         # 16 k-tiles
DT = D // P         # 8 d-tiles
EXP_SCALE = 1.0 / 32768.0  # 1/(32*D**0.5*32): q,k carry 32x each
W_SCALE = 1.0

FP8 = mybir.dt.float8e4
BF16 = mybir.dt.bfloat16
F32 = mybir.dt.float32
AF = mybir.ActivationFunctionType
DR = mybir.MatmulPerfMode.DoubleRow
ADD = mybir.AluOpType.add

TRACE = False
LAST_RESULT = None
N_WARM = 5
N_BRIDGE = 2


def enable_trace():
    """Register the NTFF profile hook that the agent image's antenv lacks,
    and neuter the artifact upload (no bucket in this container)."""
    global TRACE
    import types

    import antenv
    import concourse.bass_utils as bu

    if "antenv.axon_hooks" not in sys.modules:
        mod = types.ModuleType("antenv.axon_hooks")
        mod._hook = None
        mod.set_axon_ntff_profile_hook = lambda h: setattr(mod, "_hook", h)
        mod.get_axon_ntff_profile_hook = lambda: mod._hook
        sys.modules["antenv.axon_hooks"] = mod
        antenv.axon_hooks = mod
    from trn_agent_boot.trn_boot import _ntff_profile_via_ctypes

    sys.modules["antenv.axon_hooks"].set_axon_ntff_profile_hook(
        _ntff_profile_via_ctypes("/opt/axon/libaxon_pjrt.so")
    )
    bu.upload_artifacts = lambda tmpdir: tmpdir
    TRACE = True


def build_bass():
    nc = bacc.Bacc("TRN2", target_bir_lowering=False, debug=False, num_devices=B)
    # block-major swizzle: *_d[b, p, t*QB + s] = x[b*QB + s, t*P + p]
    idxT_d = nc.declare_dram_parameter("idxT", [NB, P, DT * QB], BF16, isOutput=False)
    idx8_d = nc.declare_dram_parameter("idx8", [NB, P, DT * QB], FP8, isOutput=False)
    wqk8_d = nc.declare_dram_parameter("wqk8", [P, DT, 2 * H], FP8, isOutput=False)
    wv_d = nc.declare_dram_parameter("wv", [P, DT, H], BF16, isOutput=False)
    # oT layout: out[h, q]; row 64 holds Z = sum(exp)
    out_d = nc.declare_dram_parameter("out", [H + 1, S], BF16, isOutput=True)

    with tile.TileContext(nc) as tc:
        with (
            tc.tile_pool(name="consts", bufs=1) as consts,
            tc.tile_pool(name="data", bufs=1) as data,
            tc.tile_pool(name="ps_mm", bufs=2, space="PSUM") as ps_mm,
            tc.tile_pool(name="ps_s", bufs=2, space="PSUM") as ps_s,
            tc.tile_pool(name="ps_o", bufs=2, space="PSUM") as ps_o,
        ):
            # ---------------- warmup ----------------
            # dummy matmuls keep the PE HAM clock-gate warming up while the
            # input DMAs stream in; results are never read
            junk = consts.tile([P, QB], BF16)
            nc.gpsimd.memset(junk[:], 0.0)
            wps = ps_o.tile([P, QB // 2], F32, tag="po", name="warm")

            def junk_mm(n, cols=QB // 2):
                # HAM duty filler: dep-free matmuls into the warmup PSUM.
                # Only legal before the second ps_s score-tile allocation.
                for _ in range(n):
                    nc.tensor.matmul(
                        wps[:, 0:cols], junk[:, 0:P], junk[:, 0:cols],
                        start=True, stop=True,
                    )

            junk_mm(N_WARM, cols=QB // 2)

            # ---------------- DMAs (all on the sync HWDGE queue) ----------
            # block-major SBUF: [p][b][t][s] - per-partition contiguous 8KB
            # per block, so each block DMA is 128 large descriptors
            idxT_sb = data.tile([P, NB, DT, QB], BF16)
            idx8_sb = data.tile([P, NB, DT, QB], FP8)

            def load_blocks(sb, dram, b0, nb, ts=slice(0, DT), eng=None):
                (eng or nc.sync).dma_start(
                    sb[:, b0 : b0 + nb, ts, :],
                    dram[b0 : b0 + nb]
                    .rearrange("b p (t s) -> p b t s", t=DT)[:, :, ts, :],
                )

            # sync queue ordered by consumer deadline
            wqk8_sb = consts.tile([P, DT, 2 * H], FP8)
            nc.sync.dma_start(wqk8_sb[:], wqk8_d[:])
            load_blocks(idx8_sb, idx8_d, 0, 1, slice(0, 4))
            load_blocks(idx8_sb, idx8_d, 0, 1, slice(4, 8))
            load_blocks(idx8_sb, idx8_d, 1, 1)
            wv_sb = consts.tile([P, DT, H], BF16)
            nc.sync.dma_start(wv_sb[:], wv_d[:])
            load_blocks(idxT_sb, idxT_d, 0, 1, slice(0, 4))
            load_blocks(idxT_sb, idxT_d, 0, 1, slice(4, 8))
            load_blocks(idx8_sb, idx8_d, 2, 1)
            load_blocks(idxT_sb, idxT_d, 1, 1)
            load_blocks(idx8_sb, idx8_d, 3, 1)
            load_blocks(idxT_sb, idxT_d, 2, 1)
            load_blocks(idxT_sb, idxT_d, 3, 1)

            # ---------------- constants (gpsimd, hidden in DMA wait) ------
            ident = consts.tile([P, P], BF16)
            masks.make_identity(nc, ident[:])
            # permutation that swaps partition halves: perm[k, p] = 1 iff
            # p == (k + 64) % 128
            perm = consts.tile([P, P], BF16)
            nc.gpsimd.memset(perm[:], 0.0)
            for base in (-H, H):
                nc.gpsimd.affine_select(
                    out=perm[:],
                    in_=perm[:],
                    compare_op=mybir.AluOpType.not_equal,
                    fill=1.0,
                    base=base,
                    pattern=[[1, P]],
                    channel_multiplier=-1,
                )

            # ---------------- working tiles ----------------
            qkT_sb = data.tile([P, S], BF16)   # rows 0-63 qT, rows 64-127 kT
            qkT2_sb = data.tile([P, S], BF16)  # rows 0-63 kT, rows 64-127 qT
            vT_sb = data.tile([H, S], BF16)
            v_sb = data.tile([P, KT, H + 1], BF16)  # [k, 64 v | 1.0]
            # p tiles keyed (parity, m): parity = qb & 1; qb and qb+2 reuse
            p_all = data.tile([P, 16, 2 * QB], BF16)
            oT_sb = data.tile([H + 1, S], BF16)

            # ones column of v_aug (flash-style Z accumulator row)
            nc.gpsimd.memset(v_sb[:, :, H : H + 1].rearrange("p t o -> p (t o)"), 1.0)

            # Zero the fully-masked rectangular regions of first-use diagonal
            # p tiles once; exp never writes them, AV reads them as zero.
            # First user qb of tile (par, m): diag pairs are m=2qb, 2qb+1 with
            # strip rects [512:640) and [0:256)+[512:896) respectively.
            for par, qb in ((0, 0), (0, 2), (1, 1), (1, 3)):
                j0 = par * 8 + 2 * qb
                nc.gpsimd.memset(p_all[:, j0, QB : QB + P], 0.0)
                nc.gpsimd.memset(p_all[:, j0 + 1, 0 : 2 * P], 0.0)
                nc.gpsimd.memset(p_all[:, j0 + 1, QB : QB + 3 * P], 0.0)

            # ---------------- projections ----------------
            def proj_qk_part(b, ts, state):
                cols = slice(b * QB, (b + 1) * QB)
                if ts.start == 0:
                    state[b] = ps_mm.tile([P, QB], F32, tag="mm", name=f"qk_{b}")
                ps = state[b]
                for t in range(ts.start, ts.stop, 2):
                    nc.tensor.matmul(
                        ps[:],
                        wqk8_sb[:, t : t + 2, :],
                        idx8_sb[:, b, t : t + 2, :],
                        start=(t == 0),
                        stop=(t == DT - 2),
                        perf_mode=DR,
                        skip_group_check=True,
                    )
                if ts.stop == DT:
                    nc.vector.tensor_copy(qkT_sb[:, cols], ps[:])

            _qk_state = {}

            def proj_qk(b):
                proj_qk_part(b, slice(0, DT), _qk_state)


            def swap(b):
                cols = slice(b * QB, (b + 1) * QB)
                ps = ps_mm.tile([P, QB], F32, tag="mm", name=f"perm_{b}")
                nc.tensor.matmul(
                    ps[:], perm[:], qkT_sb[:, cols], start=True, stop=True
                )
                nc.vector.tensor_copy(qkT2_sb[:, cols], ps[:])

            _v_state = {}

            def proj_v_part(b, ts):
                # 2-way col-tiled over the q (moving) dim; both halves run
                # concurrently in the PE array (cols 0-63 vs 64-127)
                if ts.start == 0:
                    _v_state[b] = ps_mm.tile(
                        [P, QB // 2], F32, tag="mm", name=f"vps_{b}"
                    )
                ps = _v_state[b]
                hq = QB // 2
                for t in range(ts.start, ts.stop):
                    for half in (0, 1):
                        nc.tensor.matmul(
                            ps[half * H : (half + 1) * H, :],
                            wv_sb[:, t, :],
                            idxT_sb[:, b, t, half * hq : (half + 1) * hq],
                            start=(t == 0),
                            stop=(t == DT - 1),
                            tile_position=(0, half * H),
                            skip_group_check=True,
                        )
                if ts.stop == DT:
                    # two casts reassemble the flat vT row-block: psum rows
                    # 0:64 hold q-cols [0,256), rows 64:128 hold [256,512)
                    c0 = b * QB
                    nc.vector.tensor_copy(vT_sb[:, c0 : c0 + hq], ps[0:H, :])
                    nc.vector.tensor_copy(
                        vT_sb[:, c0 + hq : c0 + QB], ps[H:P, :]
                    )

            def v_transpose_group(g):
                # tiles 4g..4g+3: v natural layout [k, 64] via PE transpose
                pst = ps_mm.tile([P, 4, H], BF16, tag="mm", name=f"vt_{g}")
                for u in range(4):
                    j = 4 * g + u
                    nc.tensor.matmul(
                        pst[:, u, :],
                        vT_sb[:, j * P : (j + 1) * P],
                        ident[:H, :H],
                        is_transpose=True,
                        start=(u == 0),
                        stop=(u == 3),
                        skip_group_check=True,
                    )
                nc.vector.tensor_copy(v_sb[:, 4 * g : 4 * g + 4, 0:H], pst[:])

            # ---------------- attention ----------------
            def attention_pairs(qb, po):
                """One (qb, k-tile-pair) step per yield so streams interleave."""
                par = qb & 1
                last_m = 2 * qb + 1

                def emit_av(m):
                    ia, ib = 2 * m, 2 * m + 1
                    offa = max(0, P * ia - QB * qb)
                    offb = max(0, P * ib - QB * qb)
                    p_ap = p_all[:, par * 8 + m, :]
                    nc.tensor.matmul(
                        po[:, offa:],
                        v_sb[:, ia, :],
                        p_ap[:, offa:QB],
                        start=(m == 0),
                        stop=False,
                        skip_group_check=True,
                    )
                    nc.tensor.matmul(
                        po[:, offb:],
                        v_sb[:, ib, :],
                        p_ap[:, QB + offb : 2 * QB],
                        start=False,
                        stop=(m == last_m),
                        skip_group_check=True,
                    )

                for m in range(2 * qb + 2):
                    ia, ib = 2 * m, 2 * m + 1
                    offa = max(0, P * ia - QB * qb)
                    offb = max(0, P * ib - QB * qb)
                    diag = m >= 2 * qb
                    ps = ps_s.tile([P, 2 * QB], F32, tag="s")
                    nc.tensor.matmul(
                        ps[:, offa:QB],
                        qkT2_sb[0:H, ia * P : (ia + 1) * P],
                        qkT_sb[0:H, qb * QB + offa : (qb + 1) * QB],
                        start=True,
                        stop=True,
                    )
                    nc.tensor.matmul(
                        ps[:, QB + offb : 2 * QB],
                        qkT_sb[H:P, ib * P : (ib + 1) * P],
                        qkT2_sb[H:P, qb * QB + offb : (qb + 1) * QB],
                        start=True,
                        stop=True,
                    )
                    p_ap = p_all[:, par * 8 + m, :]
                    if diag:
                        nc.scalar.activation(
                            p_ap[:, offa:QB],
                            ps[:, offa:QB],
                            AF.Exp,
                            scale=EXP_SCALE,
                        )
                        nc.scalar.activation(
                            p_ap[:, QB + offb : 2 * QB],
                            ps[:, QB + offb : 2 * QB],
                            AF.Exp,
                            scale=EXP_SCALE,
                        )
                        # zero the upper triangle of each diagonal 128x128 tile
                        for r0 in (offa, QB + offb):
                            nc.gpsimd.affine_select(
                                out=p_ap[:, r0 : r0 + P],
                                in_=p_ap[:, r0 : r0 + P],
                                compare_op=mybir.AluOpType.is_ge,
                                fill=0.0,
                                base=0,
                                pattern=[[1, P]],
                                channel_multiplier=-1,
                            )
                    else:
                        nc.scalar.activation(
                            p_ap[:, :], ps[:, :], AF.Exp, scale=EXP_SCALE
                        )
                    if m > 1:
                        emit_av(m - 2)
                    yield
                emit_av(last_m - 1)
                emit_av(last_m)

            def attention_epilogue(qb, po):
                cols = slice(qb * QB, (qb + 1) * QB)
                nc.vector.tensor_copy(oT_sb[:, cols], po[:])
                nc.sync.dma_start(out_d[:, cols], oT_sb[:, cols])

            def attention_blocks(qbs, bg):
                pos = {
                    qb: ps_o.tile([H + 1, QB], F32, tag="po", name=f"po_{qb}")
                    for qb in qbs
                }
                gens = [(qb, attention_pairs(qb, pos[qb])) for qb in qbs]
                bg = list(bg)
                while gens:
                    nxt = []
                    for qb, g in gens:
                        try:
                            next(g)
                            nxt.append((qb, g))
                            if bg:
                                bg.pop(0)()
                        except StopIteration:
                            attention_epilogue(qb, pos[qb])
                    gens = nxt
                for fn in bg:
                    fn()

            # ---------------- schedule ----------------
            proj_qk(0)
            junk_mm(3, cols=P * 2)
            swap(0)
            junk_mm(3, cols=P * 2)
            proj_qk(1)
            swap(1)
            junk_mm(N_BRIDGE, cols=P * 2)
            bg01 = [
                lambda: proj_v_part(0, slice(0, 8)),
                lambda: v_transpose_group(0),
                lambda: proj_qk_part(2, slice(0, 4), _qk_state),
                lambda: proj_qk_part(2, slice(4, 8), _qk_state),
                lambda: swap(2),
                lambda: proj_v_part(1, slice(0, 8)),
                lambda: v_transpose_group(1),
                lambda: proj_qk_part(3, slice(0, 4), _qk_state),
                lambda: proj_qk_part(3, slice(4, 8), _qk_state),
                lambda: swap(3),
            ]
            attention_blocks([0, 1], bg01)
            bg23 = [
                lambda: proj_v_part(2, slice(0, 8)),
                lambda: v_transpose_group(2),
                lambda: proj_v_part(3, slice(0, 8)),
                lambda: v_transpose_group(3),
            ]
            attention_blocks([2, 3], bg23)
    nc.compile()
    return nc


_NC = None


def _get_nc():
    global _NC
    if _NC is None:
        _NC = build_bass()
    return _NC


def _fp8(x):
    return np.clip(x, -240.0, 240.0).astype(ml_dtypes.float8_e4m3)


def _swizzle(x):
    # [S, D] -> [NB, P, DT*QB] with [b, p, t*QB + s] = x[b*QB + s, t*P + p]
    xT = np.ascontiguousarray(x.T)  # [D, S]
    return np.ascontiguousarray(
        xT.reshape(DT, P, NB, QB).transpose(2, 1, 0, 3).reshape(NB, P, DT * QB)
    )


def kernel(idx, Wk, Wq, Wv):
    global LAST_RESULT
    idx = np.asarray(idx, dtype=np.float32)
    Wk = np.asarray(Wk, dtype=np.float32)
    Wq = np.asarray(Wq, dtype=np.float32)
    Wv = np.asarray(Wv, dtype=np.float32)

    wqk8 = _fp8(
        np.ascontiguousarray(
            (np.concatenate([Wq, Wk], axis=1) * 32.0)
            .reshape(DT, P, 2 * H)
            .transpose(1, 0, 2)
        )
    )
    wv = np.ascontiguousarray(
        Wv.reshape(DT, P, H).transpose(1, 0, 2)
    ).astype(ml_dtypes.bfloat16)
    in_maps = []
    for i in range(B):
        xs = _swizzle(idx[i])
        in_maps.append(
            {
                "idxT": xs.astype(ml_dtypes.bfloat16),
                "idx8": _fp8(xs),
                "wqk8": wqk8,
                "wv": wv,
            }
        )

    res = run_bass_kernel_spmd(_get_nc(), in_maps, core_ids=list(range(B)), trace=TRACE)
    LAST_RESULT = res

    out = np.empty((B, S, H), dtype=np.float32)
    for i in range(B):
        o = np.asarray(res.results[i]["out"], dtype=np.float32)  # [65, S]
        out[i] = (o[:H, :] / o[H, :]).T
    return out


if __name__ == "__main__":
    rng = np.random.default_rng(0)
    idx = rng.standard_normal((B, S, D), dtype=np.float32)
    Wk = rng.standard_normal((D, H), dtype=np.float32) / np.sqrt(D)
    Wq = rng.standard_normal((D, H), dtype=np.float32) / np.sqrt(D)
    Wv = rng.standard_normal((D, H), dtype=np.float32) / np.sqrt(D)
    o = kernel(idx=idx, Wk=Wk, Wq=Wq, Wv=Wv)
    print(o.shape, o.dtype, np.abs(o).mean())
